# revision 35
# baseline (speedup 1.0000x reference)
"""Trainium2 Bass kernel for nn_ODE_71743133713072 (v2).

Semantics (unrolled from the reference lax.scan; time_steps = linspace, all
nonzero gaps equal h = ts[1]-ts[0]):
  out[:, 0]   = lat[:, 0]
  out[:, 2]   = lat[:, 1]                     (the scan's zero-length gap)
  out[:, t+1] = lat[:, t] + h * f(lat[:, t])  for t = 0..99, t != 1
  y = out[:, 100]
  out[:, k+1] = y = y + h * f(y)              for k = 100..118
where f is the D->U->U->D tanh MLP.

v2 layout strategy: everything on device lives FEATURE-ON-PARTITION
(transposed). The host pre-transposes the latents into
  xT8  [128p, 2dc, 100t, 128b] fp8   (matmul operand, unscaled)
  xT16 [128p, 2dc, 100t, 128b] fp16  (Euler base, with h*b3 pre-added)
and the device writes the output transposed (oT [128p, 2dc, 120t, 128b]
f32); the host de-transposes after gather. This removes every PE
transpose, every on-device cast, and every bias-seed matmul from the
parallel part: biases b1/b2 ride along as per-partition act biases, and
b3*h is folded into xT16. Matmuls are fp8 DoubleRow (K=256 in one pass,
0.5 cycles/col); weight loads pipeline behind the previous matmul.

The 19-step prediction chain keeps a transposed f32 carry, uses fp8-DR
for all three layers with tiny K=2 PSUM bias seeds, and is interleaved
between the parallel groups so its serial latency hides behind
DMA-bound group work.
"""

import os
import sys
from contextlib import ExitStack

import numpy as np

for _p in ("/opt/trn_rl_repo", "/root/.axon_site/_ro/trn_rl_repo"):
    if os.path.isdir(_p) and _p not in sys.path:
        sys.path.append(_p)

import ml_dtypes  # noqa: E402

B, T_OBS, KPRED, D = 1024, 100, 20, 256
T = T_OBS + KPRED          # 120
NCORES = 8
PB = B // NCORES           # 128 rows per core
P = 128
W = 8                      # frames per full group
NG_FULL = 12               # 12 full groups of 8 = 96 frames
W_LAST = 4                 # +1 group of 4 (frames 96..99)
NSTEPS = T - 1 - T_OBS     # 19 chain steps


def _emit(ctx, tc, xT8d, xT16d, w8d, bactd, bseedd, ones8d, w16d, b16d,
          oTd, h):
    import concourse.mybir as mybir

    nc = tc.nc
    F32 = mybir.dt.float32
    FP16 = mybir.dt.float16
    FP8 = mybir.dt.float8e4
    AF = mybir.ActivationFunctionType
    ALU = mybir.AluOpType
    DR = mybir.MatmulPerfMode.DoubleRow

    h8 = float(h / 8.0)

    const = ctx.enter_context(tc.tile_pool(name="const", bufs=1))
    w8 = const.tile([P, 3, 2, D], FP8, tag="w8")
    nc.sync.dma_start(w8[:], w8d[:])
    bact = const.tile([P, 4], F32, tag="bact")
    nc.sync.dma_start(bact[:], bactd[:])
    bseed = const.tile([1, 2, 3, D], FP8, tag="bseed")
    nc.sync.dma_start(bseed[:], bseedd[:])
    ones8 = const.tile([1, 2, P], FP8, tag="ones8")
    nc.sync.dma_start(ones8[:], ones8d[:])
    w16 = const.tile([P, 2, D], FP16, tag="w16")
    nc.sync.dma_start(w16[:], w16d[:])
    b16 = const.tile([1, D + P], FP16, tag="b16")  # [b1 (256) | ones (128)]
    nc.sync.dma_start(b16[:], b16d[:])

    x16p = ctx.enter_context(tc.tile_pool(name="x16", bufs=3))
    x8p = ctx.enter_context(tc.tile_pool(name="x8", bufs=3))
    hp = ctx.enter_context(tc.tile_pool(name="hact", bufs=4))
    oTp = ctx.enter_context(tc.tile_pool(name="oT", bufs=3))
    psp = ctx.enter_context(tc.tile_pool(name="ps", bufs=3, space="PSUM"))

    hcp = ctx.enter_context(tc.tile_pool(name="hc", bufs=4))
    collp = ctx.enter_context(tc.tile_pool(name="coll", bufs=2))
    chps = ctx.enter_context(tc.tile_pool(name="chps", bufs=2, space="PSUM"))

    def g_load(g):
        """DMA-in for group g; returns (x16, x8)."""
        w = W if g < NG_FULL else W_LAST
        t0 = g * W
        x16 = x16p.tile([P, 2, W, P], FP16, tag="x16")
        nc.sync.dma_start(x16[:, :, 0:w, :], xT16d[:, :, t0:t0 + w, :])
        x8 = x8p.tile([P, 2, W, P], FP8, tag="x8")
        nc.sync.dma_start(x8[:, :, 0:w, :], xT8d[:, :, t0:t0 + w, :])
        return x16, x8

    def g_layer(g, wi, rhs_of, out_fp8):
        """One MLP layer for group g: 2mc x w/4 matmuls + per-mc tanh."""
        w = W if g < NG_FULL else W_LAST
        nw = w * P
        mm = [psp.tile([P, W * P], F32, tag="ps", name="mm") for _ in range(2)]
        for mc in range(2):
            for q in range(w // 4):
                nc.tensor.matmul(
                    mm[mc][:, q * 512:(q + 1) * 512],
                    w8[:, wi, :, mc * P:(mc + 1) * P],
                    rhs_of(q), start=True, stop=True, perf_mode=DR)
        if out_fp8 is not None:
            for mc in range(2):
                nc.scalar.activation(out_fp8[:, mc, 0:nw], mm[mc][:, 0:nw],
                                     AF.Tanh, bias=bact[:, 2 * wi + mc:
                                                        2 * wi + mc + 1],
                                     scale=0.125)
        return mm

    def g_store(g, fT, x16):
        """Euler update + DMA-out for group g."""
        w = W if g < NG_FULL else W_LAST
        t0 = g * W
        nw = w * P
        oT = oTp.tile([P, 2, W, P], FP16, tag="oT")
        for mc in range(2):
            nc.vector.scalar_tensor_tensor(
                oT[:, mc, 0:w, :].rearrange("p a b -> p (a b)"),
                fT[mc][:, 0:nw], h8,
                x16[:, mc, 0:w, :].rearrange("p a b -> p (a b)"),
                ALU.mult, ALU.add)
        nc.sync.dma_start(oTd[:, :, t0 + 1:t0 + w + 1, :], oT[:, :, 0:w, :])
        if g == NG_FULL:
            # chain carry init: y0 = out[:, 100] (already fp16)
            ch["prev"] = oT[:, :, W_LAST - 1, :]
        return oT

    loads = {}

    def group(g, mid1=None, mid2=None, prefetch=None, pend=None):
        """Group slot. With `pend`, L3+store of the previous group runs here
        (between L1 and L2, where it is dependency-free), and this group's
        own L3 closure is returned via pend for the next slot."""
        x16, x8 = loads.pop(g, None) or g_load(g)
        h1 = hp.tile([P, 2, W * P], FP8, tag="h1")
        h2 = hp.tile([P, 2, W * P], FP8, tag="h2")
        g_layer(g, 0, lambda q: x8[:, :, 4 * q:4 * q + 4, :], h1)
        if prefetch is not None and prefetch not in loads:
            loads[prefetch] = g_load(prefetch)
        if pend is not None and pend["l3"] is not None:
            pend["l3"]()
        if mid1 is not None:
            mid1()
        g_layer(g, 1, lambda q: h1[:, :, q * 512:(q + 1) * 512], h2)
        if mid2 is not None:
            mid2()
        if pend is None:
            fT = g_layer(g, 2, lambda q: h2[:, :, q * 512:(q + 1) * 512], None)
            return g_store(g, fT, x16)

        def do_l3(g=g, h2=h2, x16=x16):
            fT = g_layer(g, 2, lambda q: h2[:, :, q * 512:(q + 1) * 512], None)
            g_store(g, fT, x16)
        pend["l3"] = do_l3
        return None

    # chain state: carry slices; coll tiles batch 4 output frames per DMA
    ch = {"prev": None, "coll": None}

    def chain_step(k):
        """out[:,100+k+1] = y + h*f(y); y is the transposed fp16 carry."""
        ytp = ch["prev"]   # [P, 2, P] fp16
        # L1 in fp16 straight off the carry (no cast hop): exact b1 seed
        # (K=1) + two kc-accumulated fp16 matmuls per mc chunk.
        c1 = chps.tile([P, 2, P], F32, tag="chp", name="c1")
        for mc in range(2):
            nc.tensor.matmul(c1[:, mc, :], b16[:, mc * P:(mc + 1) * P],
                             b16[:, D:D + P], start=True, stop=False)
        for mc in range(2):
            for kc in range(2):
                nc.tensor.matmul(c1[:, mc, :],
                                 w16[:, kc, mc * P:(mc + 1) * P],
                                 ytp[:, kc, :], start=False, stop=(kc == 1))
        h1c = hcp.tile([P, 2, P], FP8, tag="h1c")
        nc.scalar.activation(h1c[:], c1[:], AF.Tanh)
        c2 = chps.tile([P, 2, P], F32, tag="chp", name="c2")
        for mc in range(2):
            nc.tensor.matmul(c2[:, mc, :], bseed[:, :, 1, mc * P:(mc + 1) * P],
                             ones8[:], start=True, stop=False, perf_mode=DR)
        for mc in range(2):
            nc.tensor.matmul(c2[:, mc, :], w8[:, 1, :, mc * P:(mc + 1) * P],
                             h1c[:], start=False, stop=True, perf_mode=DR)
        h2c = hcp.tile([P, 2, P], FP8, tag="h2c")
        nc.scalar.activation(h2c[:], c2[:], AF.Tanh, scale=0.125)
        c3 = chps.tile([P, 2, P], F32, tag="chp", name="c3")
        for mc in range(2):
            nc.tensor.matmul(c3[:, mc, :], bseed[:, :, 2, mc * P:(mc + 1) * P],
                             ones8[:], start=True, stop=False, perf_mode=DR)
        for mc in range(2):
            nc.tensor.matmul(c3[:, mc, :], w8[:, 2, :, mc * P:(mc + 1) * P],
                             h2c[:], start=False, stop=True, perf_mode=DR)
        j = k % 4
        if j == 0:
            ch["coll"] = collp.tile([P, 2, 4, P], FP16, tag="coll",
                                    name="coll")
        coll = ch["coll"]
        ytn = coll[:, :, j, :]
        nc.vector.scalar_tensor_tensor(ytn, c3[:], h8, ytp,
                                       ALU.mult, ALU.add)
        ch["prev"] = ytn
        if j == 3 or k == NSTEPS - 1:
            t0 = T_OBS + 1 + (k // 4) * 4
            nc.sync.dma_start(oTd[:, :, t0:t0 + j + 1, :], coll[:, :, 0:j + 1, :])

    # ---- schedule: PE warmup, then one software-pipelined slot sequence.
    # Slot order starts with the W=4 group (frames 96-99) since the chain
    # hangs off its last output; each slot runs the previous group's L3+store
    # between its L1 and L2 (dependency-free there), with chain steps filling
    # the two act-latency windows. ----
    warm = chps.tile([P, 2, P], F32, tag="chp", name="warm")
    for i in range(10):
        nc.tensor.matmul(warm[:, i % 2, :], bseed[:, :, 0, 0:P], ones8[:],
                         start=True, stop=True, perf_mode=DR)
    wsink = hcp.tile([P, 2, P], FP8, tag="wsink")
    nc.scalar.activation(wsink[:], warm[:], AF.Tanh)

    state = {"k": 0}

    def fill():
        if ch["prev"] is not None and state["k"] < NSTEPS:
            chain_step(state["k"])
            state["k"] += 1

    group(NG_FULL, prefetch=0)            # frames 96..99 -> out 97..100
    pend = {"l3": None}
    for g in range(NG_FULL):
        group(g, mid1=fill, mid2=fill,
              prefetch=g + 1 if g + 1 < NG_FULL else None, pend=pend)
    pend["l3"]()
    while state["k"] < NSTEPS:
        fill()


def _build(h):
    import concourse.mybir as mybir
    import concourse.tile as tile
    from concourse import bacc

    F32 = mybir.dt.float32
    FP16 = mybir.dt.float16
    FP8 = mybir.dt.float8e4

    nc = bacc.Bacc("TRN2", target_bir_lowering=False, debug=False,
                   num_devices=NCORES)
    xT8d = nc.dram_tensor("xT8", [P, 2, T_OBS, P], FP8,
                          kind="ExternalInput").ap()
    xT16d = nc.dram_tensor("xT16", [P, 2, T_OBS, P], FP16,
                           kind="ExternalInput").ap()
    w8d = nc.dram_tensor("w8", [P, 3, 2, D], FP8, kind="ExternalInput").ap()
    bactd = nc.dram_tensor("bact", [P, 4], F32, kind="ExternalInput").ap()
    bseedd = nc.dram_tensor("bseed", [1, 2, 3, D], FP8,
                            kind="ExternalInput").ap()
    ones8d = nc.dram_tensor("ones8", [1, 2, P], FP8, kind="ExternalInput").ap()
    w16d = nc.dram_tensor("w16", [P, 2, D], FP16, kind="ExternalInput").ap()
    b16d = nc.dram_tensor("b16", [1, D + P], FP16, kind="ExternalInput").ap()
    oTd = nc.dram_tensor("oT", [P, 2, T, P], FP16, kind="ExternalOutput").ap()

    with tile.TileContext(nc) as tc, ExitStack() as ctx:
        _emit(ctx, tc, xT8d, xT16d, w8d, bactd, bseedd, ones8d, w16d, b16d,
              oTd, h)
    nc.compile()
    return nc


def _host_inputs(inputs):
    """Shared (weights/bias) device arrays + h. Returns (h, shared)."""
    ts = np.asarray(inputs["time_steps"], np.float32)
    h = float(np.float32(ts[1]) - np.float32(ts[0]))

    f8 = ml_dtypes.float8_e4m3
    W1 = np.asarray(inputs["W1"], np.float32)
    W2 = np.asarray(inputs["W2"], np.float32)
    W3 = np.asarray(inputs["W3"], np.float32)
    b1 = np.asarray(inputs["b1"], np.float32)
    b2 = np.asarray(inputs["b2"], np.float32)
    b3 = np.asarray(inputs["b3"], np.float32)

    # w8[p, wi, kc, m] = 8*W_wi[kc*128+p, m]
    w8 = np.stack([8.0 * W1, 8.0 * W2, 8.0 * W3])  # [3, 256, 256]
    w8 = w8.reshape(3, 2, P, D).transpose(2, 0, 1, 3)  # [p, 3, kc, m]
    w8 = np.ascontiguousarray(w8).astype(f8)

    bact = np.stack([b1[:P], b1[P:], b2[:P], b2[P:]], axis=1)
    bact = np.ascontiguousarray(bact.astype(np.float32))

    # bseed[0, kc, wi, m] = 4*b_wi[m]  (K=2 DR ones contraction doubles it)
    bs = np.stack([4.0 * b1, 4.0 * b2, 4.0 * b3])  # [3, 256]
    bseed = np.broadcast_to(bs[None, None], (1, 2, 3, D))
    bseed = np.ascontiguousarray(bseed).astype(f8)
    ones8 = np.ones((1, 2, P), np.float32).astype(f8)

    # chain L1 runs in fp16: exact W1/b1 (unscaled) + a ones row
    w16 = W1.reshape(2, P, D).transpose(1, 0, 2)   # [p, kc, m]
    w16 = np.ascontiguousarray(w16).astype(np.float16)
    b16 = np.concatenate([b1, np.ones(P, np.float32)]).reshape(1, D + P)
    b16 = b16.astype(np.float16)

    shared = dict(w8=w8, bact=bact, bseed=bseed, ones8=ones8,
                  w16=w16, b16=b16)
    return h, shared


def make_in_maps(inputs):
    """Full per-core input maps (shared + per-core transposed latents)."""
    h, shared = _host_inputs(inputs)
    b3 = np.asarray(inputs["b3"], np.float32)
    lat = np.ascontiguousarray(np.asarray(inputs["latents"], np.float32))
    f8 = ml_dtypes.float8_e4m3

    in_maps = []
    for c in range(NCORES):
        lc = lat[c * PB:(c + 1) * PB]                  # [128b, 100t, 256d]
        xt = lc.transpose(2, 1, 0)                     # [256d, 100t, 128b]
        xt8 = xt.reshape(2, P, T_OBS, P).transpose(1, 0, 2, 3)
        xt16 = (xt + (np.float32(h) * b3)[:, None, None])
        xt16 = xt16.reshape(2, P, T_OBS, P).transpose(1, 0, 2, 3)
        m = dict(shared)
        m["xT8"] = np.ascontiguousarray(xt8).astype(f8)
        m["xT16"] = np.ascontiguousarray(xt16).astype(np.float16)
        in_maps.append(m)
    return h, in_maps


def assemble_out(inputs, core_outs):
    """De-transpose per-core oT outputs and patch the exact copy frames."""
    lat = np.asarray(inputs["latents"], np.float32)
    out = np.empty((B, T, D), np.float32)
    for c in range(NCORES):
        oT = np.asarray(core_outs[c], np.float32)      # [128p, 2dc, 120t, 128b]
        out[c * PB:(c + 1) * PB] = oT.transpose(3, 2, 1, 0).reshape(PB, T, D)
    out[:, 0, :] = lat[:, 0, :]
    out[:, 2, :] = lat[:, 1, :]
    return out


_CACHE = {}


def kernel(**inputs):
    from concourse.bass_utils import run_bass_kernel_spmd

    h, in_maps = make_in_maps(inputs)
    if h not in _CACHE:
        _CACHE[h] = _build(h)
    nc = _CACHE[h]

    res = run_bass_kernel_spmd(nc, in_maps, list(range(NCORES)))
    outs = [res.results[c]["oT"] for c in range(NCORES)]
    return assemble_out(inputs, outs)


# revision 36
# speedup vs baseline: 1.0111x; 1.0111x over previous
"""Trainium2 Bass kernel for nn_ODE_71743133713072 (v2).

Semantics (unrolled from the reference lax.scan; time_steps = linspace, all
nonzero gaps equal h = ts[1]-ts[0]):
  out[:, 0]   = lat[:, 0]
  out[:, 2]   = lat[:, 1]                     (the scan's zero-length gap)
  out[:, t+1] = lat[:, t] + h * f(lat[:, t])  for t = 0..99, t != 1
  y = out[:, 100]
  out[:, k+1] = y = y + h * f(y)              for k = 100..118
where f is the D->U->U->D tanh MLP.

Layout strategy: everything on device lives FEATURE-ON-PARTITION
(transposed). The host pre-transposes the latents into
  xT8  [128p, 2dc, 100t, 128b] fp8   (matmul operand, unscaled)
  xT16 [128p, 2dc, 100t, 128b] fp16  (Euler base, with h*b3 pre-added)
and the device writes the output transposed AND in fp16
(oT [128p, 2dc, 120t, 128b]); the host de-transposes/upcasts after
gather. This removes every PE transpose, every on-device cast, and
every bias matmul from the parallel part (b1/b2 ride along as
per-partition act biases, b3*h is folded into xT16), and halves the
output DMA. Matmuls are fp8 DoubleRow (K=256 in one pass, 0.5
cycles/col, 216 ns/inst steady at the sustained 1.2 GHz PE clock);
LDWEIGHTS pipelines behind the previous matmul.

Scheduling (the key to keeping the PE p-state warm): groups of 8 frames
are software-pipelined — each slot runs L1(g), then the PREVIOUS
group's L3+Euler+store (dependency-free here), then L2(g), with one
19-step-chain step embedded in each of the two act-latency windows so
the tensor queue never drains. The chain keeps a transposed fp16 carry
(the collector slice doubles as carry and DMA source), runs L1 in fp16
straight off the carry (no cast hop), L2/L3 in fp8-DR with K<=2 PSUM
bias seeds, and batches 4 output frames per DMA.

PSUM budget (8 banks): 3 rotating [128,1024]-f32 matmul tiles (6) +
2 chain tiles (2). Measured: ~106-110 us on 8 cores (baseline 232 us),
rel err 1.5e-3 (fp8 weights + fp16 carry/out, tolerance 2e-2).
"""

import os
import sys
from contextlib import ExitStack

import numpy as np

for _p in ("/opt/trn_rl_repo", "/root/.axon_site/_ro/trn_rl_repo"):
    if os.path.isdir(_p) and _p not in sys.path:
        sys.path.append(_p)

import ml_dtypes  # noqa: E402

B, T_OBS, KPRED, D = 1024, 100, 20, 256
T = T_OBS + KPRED          # 120
NCORES = 8
PB = B // NCORES           # 128 rows per core
P = 128
W = 8                      # frames per full group
NG_FULL = 12               # 12 full groups of 8 = 96 frames
W_LAST = 4                 # +1 group of 4 (frames 96..99)
NSTEPS = T - 1 - T_OBS     # 19 chain steps


def _emit(ctx, tc, xT8d, xT16d, w8d, bactd, bseedd, ones8d, w16d, b16d,
          oTd, h):
    import concourse.mybir as mybir

    nc = tc.nc
    F32 = mybir.dt.float32
    FP16 = mybir.dt.float16
    FP8 = mybir.dt.float8e4
    AF = mybir.ActivationFunctionType
    ALU = mybir.AluOpType
    DR = mybir.MatmulPerfMode.DoubleRow

    h8 = float(h / 8.0)

    const = ctx.enter_context(tc.tile_pool(name="const", bufs=1))
    w8 = const.tile([P, 3, 2, D], FP8, tag="w8")
    nc.sync.dma_start(w8[:], w8d[:])
    bact = const.tile([P, 4], F32, tag="bact")
    nc.sync.dma_start(bact[:], bactd[:])
    bseed = const.tile([1, 2, 3, D], FP8, tag="bseed")
    nc.sync.dma_start(bseed[:], bseedd[:])
    ones8 = const.tile([1, 2, P], FP8, tag="ones8")
    nc.sync.dma_start(ones8[:], ones8d[:])
    w16 = const.tile([P, 2, D], FP16, tag="w16")
    nc.sync.dma_start(w16[:], w16d[:])
    b16 = const.tile([1, D + P], FP16, tag="b16")  # [b1 (256) | ones (128)]
    nc.sync.dma_start(b16[:], b16d[:])

    x16p = ctx.enter_context(tc.tile_pool(name="x16", bufs=3))
    x8p = ctx.enter_context(tc.tile_pool(name="x8", bufs=3))
    hp = ctx.enter_context(tc.tile_pool(name="hact", bufs=4))
    oTp = ctx.enter_context(tc.tile_pool(name="oT", bufs=3))
    psp = ctx.enter_context(tc.tile_pool(name="ps", bufs=3, space="PSUM"))

    hcp = ctx.enter_context(tc.tile_pool(name="hc", bufs=4))
    collp = ctx.enter_context(tc.tile_pool(name="coll", bufs=2))
    chps = ctx.enter_context(tc.tile_pool(name="chps", bufs=2, space="PSUM"))

    def g_load(g):
        """DMA-in for group g; returns (x16, x8)."""
        w = W if g < NG_FULL else W_LAST
        t0 = g * W
        x16 = x16p.tile([P, 2, W, P], FP16, tag="x16")
        nc.sync.dma_start(x16[:, :, 0:w, :], xT16d[:, :, t0:t0 + w, :])
        x8 = x8p.tile([P, 2, W, P], FP8, tag="x8")
        nc.sync.dma_start(x8[:, :, 0:w, :], xT8d[:, :, t0:t0 + w, :])
        return x16, x8

    def g_layer(g, wi, rhs_of, out_fp8):
        """One MLP layer for group g: 2mc x w/4 matmuls + per-mc tanh."""
        w = W if g < NG_FULL else W_LAST
        nw = w * P
        mm = [psp.tile([P, W * P], F32, tag="ps", name="mm") for _ in range(2)]
        for mc in range(2):
            for q in range(w // 4):
                nc.tensor.matmul(
                    mm[mc][:, q * 512:(q + 1) * 512],
                    w8[:, wi, :, mc * P:(mc + 1) * P],
                    rhs_of(q), start=True, stop=True, perf_mode=DR)
        if out_fp8 is not None:
            for mc in range(2):
                nc.scalar.activation(out_fp8[:, mc, 0:nw], mm[mc][:, 0:nw],
                                     AF.Tanh, bias=bact[:, 2 * wi + mc:
                                                        2 * wi + mc + 1],
                                     scale=0.125)
        return mm

    def g_store(g, fT, x16):
        """Euler update + DMA-out for group g."""
        w = W if g < NG_FULL else W_LAST
        t0 = g * W
        nw = w * P
        oT = oTp.tile([P, 2, W, P], FP16, tag="oT")
        for mc in range(2):
            nc.vector.scalar_tensor_tensor(
                oT[:, mc, 0:w, :].rearrange("p a b -> p (a b)"),
                fT[mc][:, 0:nw], h8,
                x16[:, mc, 0:w, :].rearrange("p a b -> p (a b)"),
                ALU.mult, ALU.add)
        nc.sync.dma_start(oTd[:, :, t0 + 1:t0 + w + 1, :], oT[:, :, 0:w, :])
        if g == NG_FULL:
            # chain carry init: y0 = out[:, 100] (already fp16)
            ch["prev"] = oT[:, :, W_LAST - 1, :]
        return oT

    loads = {}

    def group(g, mid1=None, mid2=None, prefetch=None, pend=None):
        """Group slot. With `pend`, L3+store of the previous group runs here
        (between L1 and L2, where it is dependency-free), and this group's
        own L3 closure is returned via pend for the next slot."""
        x16, x8 = loads.pop(g, None) or g_load(g)
        h1 = hp.tile([P, 2, W * P], FP8, tag="h1")
        h2 = hp.tile([P, 2, W * P], FP8, tag="h2")
        g_layer(g, 0, lambda q: x8[:, :, 4 * q:4 * q + 4, :], h1)
        if prefetch is not None and prefetch not in loads:
            loads[prefetch] = g_load(prefetch)
        if pend is not None and pend["l3"] is not None:
            pend["l3"]()
        if mid1 is not None:
            mid1()
        g_layer(g, 1, lambda q: h1[:, :, q * 512:(q + 1) * 512], h2)
        if mid2 is not None:
            mid2()
        if pend is None:
            fT = g_layer(g, 2, lambda q: h2[:, :, q * 512:(q + 1) * 512], None)
            return g_store(g, fT, x16)

        def do_l3(g=g, h2=h2, x16=x16):
            fT = g_layer(g, 2, lambda q: h2[:, :, q * 512:(q + 1) * 512], None)
            g_store(g, fT, x16)
        pend["l3"] = do_l3
        return None

    # chain state: carry slices; coll tiles batch 4 output frames per DMA
    ch = {"prev": None, "coll": None}

    def chain_step(k):
        """out[:,100+k+1] = y + h*f(y); y is the transposed fp16 carry."""
        ytp = ch["prev"]   # [P, 2, P] fp16
        # L1 in fp16 straight off the carry (no cast hop): exact b1 seed
        # (K=1) + two kc-accumulated fp16 matmuls per mc chunk.
        c1 = chps.tile([P, 2, P], F32, tag="chp", name="c1")
        for mc in range(2):
            nc.tensor.matmul(c1[:, mc, :], b16[:, mc * P:(mc + 1) * P],
                             b16[:, D:D + P], start=True, stop=False)
        for mc in range(2):
            for kc in range(2):
                nc.tensor.matmul(c1[:, mc, :],
                                 w16[:, kc, mc * P:(mc + 1) * P],
                                 ytp[:, kc, :], start=False, stop=(kc == 1))
        h1c = hcp.tile([P, 2, P], FP8, tag="h1c")
        nc.scalar.activation(h1c[:], c1[:], AF.Tanh)
        c2 = chps.tile([P, 2, P], F32, tag="chp", name="c2")
        for mc in range(2):
            nc.tensor.matmul(c2[:, mc, :], bseed[:, :, 1, mc * P:(mc + 1) * P],
                             ones8[:], start=True, stop=False, perf_mode=DR)
        for mc in range(2):
            nc.tensor.matmul(c2[:, mc, :], w8[:, 1, :, mc * P:(mc + 1) * P],
                             h1c[:], start=False, stop=True, perf_mode=DR)
        h2c = hcp.tile([P, 2, P], FP8, tag="h2c")
        nc.scalar.activation(h2c[:], c2[:], AF.Tanh, scale=0.125)
        c3 = chps.tile([P, 2, P], F32, tag="chp", name="c3")
        for mc in range(2):
            nc.tensor.matmul(c3[:, mc, :], bseed[:, :, 2, mc * P:(mc + 1) * P],
                             ones8[:], start=True, stop=False, perf_mode=DR)
        for mc in range(2):
            nc.tensor.matmul(c3[:, mc, :], w8[:, 2, :, mc * P:(mc + 1) * P],
                             h2c[:], start=False, stop=True, perf_mode=DR)
        j = k % 4
        if j == 0:
            ch["coll"] = collp.tile([P, 2, 4, P], FP16, tag="coll",
                                    name="coll")
        coll = ch["coll"]
        ytn = coll[:, :, j, :]
        nc.vector.scalar_tensor_tensor(ytn, c3[:], h8, ytp,
                                       ALU.mult, ALU.add)
        ch["prev"] = ytn
        if j == 3 or k == NSTEPS - 1:
            t0 = T_OBS + 1 + (k // 4) * 4
            nc.sync.dma_start(oTd[:, :, t0:t0 + j + 1, :], coll[:, :, 0:j + 1, :])

    # ---- schedule: PE warmup, then one software-pipelined slot sequence.
    # Slot order starts with the W=4 group (frames 96-99) since the chain
    # hangs off its last output; each slot runs the previous group's L3+store
    # between its L1 and L2 (dependency-free there), with chain steps filling
    # the two act-latency windows. ----
    warm = chps.tile([P, 2, P], F32, tag="chp", name="warm")
    for i in range(10):
        nc.tensor.matmul(warm[:, i % 2, :], bseed[:, :, 0, 0:P], ones8[:],
                         start=True, stop=True, perf_mode=DR)
    wsink = hcp.tile([P, 2, P], FP8, tag="wsink")
    nc.scalar.activation(wsink[:], warm[:], AF.Tanh)

    state = {"k": 0}

    def fill():
        if ch["prev"] is not None and state["k"] < NSTEPS:
            chain_step(state["k"])
            state["k"] += 1

    group(NG_FULL, prefetch=0)            # frames 96..99 -> out 97..100
    pend = {"l3": None}
    for g in range(NG_FULL):
        group(g, mid1=fill, mid2=fill,
              prefetch=g + 1 if g + 1 < NG_FULL else None, pend=pend)
    pend["l3"]()
    while state["k"] < NSTEPS:
        fill()


def _build(h):
    import concourse.mybir as mybir
    import concourse.tile as tile
    from concourse import bacc

    F32 = mybir.dt.float32
    FP16 = mybir.dt.float16
    FP8 = mybir.dt.float8e4

    nc = bacc.Bacc("TRN2", target_bir_lowering=False, debug=False,
                   num_devices=NCORES)
    xT8d = nc.dram_tensor("xT8", [P, 2, T_OBS, P], FP8,
                          kind="ExternalInput").ap()
    xT16d = nc.dram_tensor("xT16", [P, 2, T_OBS, P], FP16,
                           kind="ExternalInput").ap()
    w8d = nc.dram_tensor("w8", [P, 3, 2, D], FP8, kind="ExternalInput").ap()
    bactd = nc.dram_tensor("bact", [P, 4], F32, kind="ExternalInput").ap()
    bseedd = nc.dram_tensor("bseed", [1, 2, 3, D], FP8,
                            kind="ExternalInput").ap()
    ones8d = nc.dram_tensor("ones8", [1, 2, P], FP8, kind="ExternalInput").ap()
    w16d = nc.dram_tensor("w16", [P, 2, D], FP16, kind="ExternalInput").ap()
    b16d = nc.dram_tensor("b16", [1, D + P], FP16, kind="ExternalInput").ap()
    oTd = nc.dram_tensor("oT", [P, 2, T, P], FP16, kind="ExternalOutput").ap()

    with tile.TileContext(nc) as tc, ExitStack() as ctx:
        _emit(ctx, tc, xT8d, xT16d, w8d, bactd, bseedd, ones8d, w16d, b16d,
              oTd, h)
    nc.compile()
    return nc


def _host_inputs(inputs):
    """Shared (weights/bias) device arrays + h. Returns (h, shared)."""
    ts = np.asarray(inputs["time_steps"], np.float32)
    h = float(np.float32(ts[1]) - np.float32(ts[0]))

    f8 = ml_dtypes.float8_e4m3
    W1 = np.asarray(inputs["W1"], np.float32)
    W2 = np.asarray(inputs["W2"], np.float32)
    W3 = np.asarray(inputs["W3"], np.float32)
    b1 = np.asarray(inputs["b1"], np.float32)
    b2 = np.asarray(inputs["b2"], np.float32)
    b3 = np.asarray(inputs["b3"], np.float32)

    # w8[p, wi, kc, m] = 8*W_wi[kc*128+p, m]
    w8 = np.stack([8.0 * W1, 8.0 * W2, 8.0 * W3])  # [3, 256, 256]
    w8 = w8.reshape(3, 2, P, D).transpose(2, 0, 1, 3)  # [p, 3, kc, m]
    w8 = np.ascontiguousarray(w8).astype(f8)

    bact = np.stack([b1[:P], b1[P:], b2[:P], b2[P:]], axis=1)
    bact = np.ascontiguousarray(bact.astype(np.float32))

    # bseed[0, kc, wi, m] = 4*b_wi[m]  (K=2 DR ones contraction doubles it)
    bs = np.stack([4.0 * b1, 4.0 * b2, 4.0 * b3])  # [3, 256]
    bseed = np.broadcast_to(bs[None, None], (1, 2, 3, D))
    bseed = np.ascontiguousarray(bseed).astype(f8)
    ones8 = np.ones((1, 2, P), np.float32).astype(f8)

    # chain L1 runs in fp16: exact W1/b1 (unscaled) + a ones row
    w16 = W1.reshape(2, P, D).transpose(1, 0, 2)   # [p, kc, m]
    w16 = np.ascontiguousarray(w16).astype(np.float16)
    b16 = np.concatenate([b1, np.ones(P, np.float32)]).reshape(1, D + P)
    b16 = b16.astype(np.float16)

    shared = dict(w8=w8, bact=bact, bseed=bseed, ones8=ones8,
                  w16=w16, b16=b16)
    return h, shared


def make_in_maps(inputs):
    """Full per-core input maps (shared + per-core transposed latents)."""
    h, shared = _host_inputs(inputs)
    b3 = np.asarray(inputs["b3"], np.float32)
    lat = np.ascontiguousarray(np.asarray(inputs["latents"], np.float32))
    f8 = ml_dtypes.float8_e4m3

    in_maps = []
    for c in range(NCORES):
        lc = lat[c * PB:(c + 1) * PB]                  # [128b, 100t, 256d]
        xt = lc.transpose(2, 1, 0)                     # [256d, 100t, 128b]
        xt8 = xt.reshape(2, P, T_OBS, P).transpose(1, 0, 2, 3)
        xt16 = (xt + (np.float32(h) * b3)[:, None, None])
        xt16 = xt16.reshape(2, P, T_OBS, P).transpose(1, 0, 2, 3)
        m = dict(shared)
        m["xT8"] = np.ascontiguousarray(xt8).astype(f8)
        m["xT16"] = np.ascontiguousarray(xt16).astype(np.float16)
        in_maps.append(m)
    return h, in_maps


def assemble_out(inputs, core_outs):
    """De-transpose per-core oT outputs and patch the exact copy frames."""
    lat = np.asarray(inputs["latents"], np.float32)
    out = np.empty((B, T, D), np.float32)
    for c in range(NCORES):
        oT = np.asarray(core_outs[c], np.float32)      # [128p, 2dc, 120t, 128b]
        out[c * PB:(c + 1) * PB] = oT.transpose(3, 2, 1, 0).reshape(PB, T, D)
    out[:, 0, :] = lat[:, 0, :]
    out[:, 2, :] = lat[:, 1, :]
    return out


_CACHE = {}


def kernel(**inputs):
    from concourse.bass_utils import run_bass_kernel_spmd

    h, in_maps = make_in_maps(inputs)
    if h not in _CACHE:
        _CACHE[h] = _build(h)
    nc = _CACHE[h]

    res = run_bass_kernel_spmd(nc, in_maps, list(range(NCORES)))
    outs = [res.results[c]["oT"] for c in range(NCORES)]
    return assemble_out(inputs, outs)


# revision 37
# speedup vs baseline: 1.0416x; 1.0301x over previous
"""Trainium2 Bass kernel for nn_ODE_71743133713072 (v2).

Semantics (unrolled from the reference lax.scan; time_steps = linspace, all
nonzero gaps equal h = ts[1]-ts[0]):
  out[:, 0]   = lat[:, 0]
  out[:, 2]   = lat[:, 1]                     (the scan's zero-length gap)
  out[:, t+1] = lat[:, t] + h * f(lat[:, t])  for t = 0..99, t != 1
  y = out[:, 100]
  out[:, k+1] = y = y + h * f(y)              for k = 100..118
where f is the D->U->U->D tanh MLP.

Layout strategy: everything on device lives FEATURE-ON-PARTITION
(transposed). The host pre-transposes the latents into
  xT8  [128p, 2dc, 100t, 128b] fp8   (matmul operand, unscaled)
  xT16 [128p, 2dc, 100t, 128b] fp16  (Euler base, with h*b3 pre-added)
and the device writes the output transposed AND in fp16
(oT [128p, 2dc, 120t, 128b]); the host de-transposes/upcasts after
gather. This removes every PE transpose, every on-device cast, and
every bias matmul from the parallel part (b1/b2 ride along as
per-partition act biases, b3*h is folded into xT16), and halves the
output DMA. Matmuls are fp8 DoubleRow (K=256 in one pass, 0.5
cycles/col, 216 ns/inst steady at the sustained 1.2 GHz PE clock);
LDWEIGHTS pipelines behind the previous matmul.

Scheduling (the key to keeping the PE p-state warm): groups of 8 frames
are software-pipelined — each slot runs L1(g), then the PREVIOUS
group's L3+Euler+store (dependency-free here), then L2(g), with one
19-step-chain step embedded in each of the two act-latency windows so
the tensor queue never drains. The chain keeps a transposed fp16 carry
(the collector slice doubles as carry and DMA source), runs L1 in fp16
straight off the carry (no cast hop), L2/L3 in fp8-DR with K<=2 PSUM
bias seeds, and batches 4 output frames per DMA.

PSUM budget (8 banks): 3 rotating [128,1024]-f32 matmul tiles (6) +
2 chain tiles (2). Measured: ~106-110 us on 8 cores (baseline 232 us),
rel err 1.5e-3 (fp8 weights + fp16 carry/out, tolerance 2e-2).
"""

import os
import sys
from contextlib import ExitStack

import numpy as np

for _p in ("/opt/trn_rl_repo", "/root/.axon_site/_ro/trn_rl_repo"):
    if os.path.isdir(_p) and _p not in sys.path:
        sys.path.append(_p)

import ml_dtypes  # noqa: E402

B, T_OBS, KPRED, D = 1024, 100, 20, 256
T = T_OBS + KPRED          # 120
NCORES = 8
PB = B // NCORES           # 128 rows per core
P = 128
W = 8                      # frames per full group
NG_FULL = 12               # 12 full groups of 8 = 96 frames
W_LAST = 4                 # +1 group of 4 (frames 96..99)
NSTEPS = T - 1 - T_OBS     # 19 chain steps


def _emit(ctx, tc, xT8d, xT16d, w8d, bactd, bseedd, ones8d, w16d, b16d,
          oTd, h):
    import concourse.mybir as mybir

    nc = tc.nc
    F32 = mybir.dt.float32
    FP16 = mybir.dt.float16
    FP8 = mybir.dt.float8e4
    AF = mybir.ActivationFunctionType
    ALU = mybir.AluOpType
    DR = mybir.MatmulPerfMode.DoubleRow

    h8 = float(h / 8.0)

    const = ctx.enter_context(tc.tile_pool(name="const", bufs=1))
    w8 = const.tile([P, 3, 2, D], FP8, tag="w8")
    nc.sync.dma_start(w8[:], w8d[:])
    bact = const.tile([P, 4], F32, tag="bact")
    nc.sync.dma_start(bact[:], bactd[:])
    bseed = const.tile([1, 2, 3, D], FP8, tag="bseed")
    nc.sync.dma_start(bseed[:], bseedd[:])
    ones8 = const.tile([1, 2, P], FP8, tag="ones8")
    nc.sync.dma_start(ones8[:], ones8d[:])
    w16 = const.tile([P, 2, D], FP16, tag="w16")
    nc.sync.dma_start(w16[:], w16d[:])
    b16 = const.tile([1, D + P], FP16, tag="b16")  # [b1 (256) | ones (128)]
    nc.sync.dma_start(b16[:], b16d[:])

    x16p = ctx.enter_context(tc.tile_pool(name="x16", bufs=3))
    x8p = ctx.enter_context(tc.tile_pool(name="x8", bufs=3))
    hp = ctx.enter_context(tc.tile_pool(name="hact", bufs=4))
    oTp = ctx.enter_context(tc.tile_pool(name="oT", bufs=3))
    psp = ctx.enter_context(tc.tile_pool(name="ps", bufs=3, space="PSUM"))

    hcp = ctx.enter_context(tc.tile_pool(name="hc", bufs=4))
    collp = ctx.enter_context(tc.tile_pool(name="coll", bufs=2))
    chps = ctx.enter_context(tc.tile_pool(name="chps", bufs=2, space="PSUM"))

    def g_load(g):
        """DMA-in for group g; returns (x16, x8)."""
        w = W if g < NG_FULL else W_LAST
        t0 = g * W
        x16 = x16p.tile([P, 2, W, P], FP16, tag="x16")
        nc.sync.dma_start(x16[:, :, 0:w, :], xT16d[:, :, t0:t0 + w, :])
        x8 = x8p.tile([P, 2, W, P], FP8, tag="x8")
        nc.sync.dma_start(x8[:, :, 0:w, :], xT8d[:, :, t0:t0 + w, :])
        return x16, x8

    def g_layer(g, wi, rhs_of, out_fp8):
        """One MLP layer for group g: 2mc x w/4 matmuls + per-mc tanh."""
        w = W if g < NG_FULL else W_LAST
        nw = w * P
        mm = [psp.tile([P, W * P], F32, tag="ps", name="mm") for _ in range(2)]
        for mc in range(2):
            for q in range(w // 4):
                nc.tensor.matmul(
                    mm[mc][:, q * 512:(q + 1) * 512],
                    w8[:, wi, :, mc * P:(mc + 1) * P],
                    rhs_of(q), start=True, stop=True, perf_mode=DR)
        if out_fp8 is not None:
            for mc in range(2):
                nc.scalar.activation(out_fp8[:, mc, 0:nw], mm[mc][:, 0:nw],
                                     AF.Tanh, bias=bact[:, 2 * wi + mc:
                                                        2 * wi + mc + 1],
                                     scale=0.125)
        return mm

    def g_store(g, fT, x16):
        """Euler update + DMA-out for group g."""
        w = W if g < NG_FULL else W_LAST
        t0 = g * W
        nw = w * P
        oT = oTp.tile([P, 2, W, P], FP16, tag="oT")
        for mc in range(2):
            nc.vector.scalar_tensor_tensor(
                oT[:, mc, 0:w, :].rearrange("p a b -> p (a b)"),
                fT[mc][:, 0:nw], h8,
                x16[:, mc, 0:w, :].rearrange("p a b -> p (a b)"),
                ALU.mult, ALU.add)
        nc.sync.dma_start(oTd[:, :, t0 + 1:t0 + w + 1, :], oT[:, :, 0:w, :])
        if g == NG_FULL:
            # chain carry init: y0 = out[:, 100] (already fp16)
            ch["prev"] = oT[:, :, W_LAST - 1, :]
        return oT

    loads = {}

    def group(g, mid1=None, mid2=None, prefetch=None, pend=None):
        """Group slot. With `pend`, L3+store of the previous group runs here
        (between L1 and L2, where it is dependency-free), and this group's
        own L3 closure is returned via pend for the next slot."""
        x16, x8 = loads.pop(g, None) or g_load(g)
        h1 = hp.tile([P, 2, W * P], FP8, tag="h1")
        h2 = hp.tile([P, 2, W * P], FP8, tag="h2")
        g_layer(g, 0, lambda q: x8[:, :, 4 * q:4 * q + 4, :], h1)
        if prefetch is not None and prefetch not in loads:
            loads[prefetch] = g_load(prefetch)
        if mid1 is not None:
            mid1()
        if pend is not None and pend["l3"] is not None:
            pend["l3"]()
        g_layer(g, 1, lambda q: h1[:, :, q * 512:(q + 1) * 512], h2)
        if mid2 is not None:
            mid2()
        if pend is None:
            fT = g_layer(g, 2, lambda q: h2[:, :, q * 512:(q + 1) * 512], None)
            return g_store(g, fT, x16)

        def do_l3(g=g, h2=h2, x16=x16):
            fT = g_layer(g, 2, lambda q: h2[:, :, q * 512:(q + 1) * 512], None)
            g_store(g, fT, x16)
        pend["l3"] = do_l3
        return None

    # chain state: carry slices; coll tiles batch 4 output frames per DMA
    ch = {"prev": None, "coll": None}

    def chain_step(k):
        """out[:,100+k+1] = y + h*f(y); y is the transposed fp16 carry."""
        ytp = ch["prev"]   # [P, 2, P] fp16
        # L1 in fp16 straight off the carry (no cast hop): exact b1 seed
        # (K=1) + two kc-accumulated fp16 matmuls per mc chunk.
        c1 = chps.tile([P, 2, P], F32, tag="chp", name="c1")
        for mc in range(2):
            nc.tensor.matmul(c1[:, mc, :], b16[:, mc * P:(mc + 1) * P],
                             b16[:, D:D + P], start=True, stop=False)
        for mc in range(2):
            for kc in range(2):
                nc.tensor.matmul(c1[:, mc, :],
                                 w16[:, kc, mc * P:(mc + 1) * P],
                                 ytp[:, kc, :], start=False, stop=(kc == 1))
        h1c = hcp.tile([P, 2, P], FP8, tag="h1c")
        nc.scalar.activation(h1c[:], c1[:], AF.Tanh)
        c2 = chps.tile([P, 2, P], F32, tag="chp", name="c2")
        for mc in range(2):
            nc.tensor.matmul(c2[:, mc, :], bseed[:, :, 1, mc * P:(mc + 1) * P],
                             ones8[:], start=True, stop=False, perf_mode=DR)
        for mc in range(2):
            nc.tensor.matmul(c2[:, mc, :], w8[:, 1, :, mc * P:(mc + 1) * P],
                             h1c[:], start=False, stop=True, perf_mode=DR)
        h2c = hcp.tile([P, 2, P], FP8, tag="h2c")
        nc.scalar.activation(h2c[:], c2[:], AF.Tanh, scale=0.125)
        c3 = chps.tile([P, 2, P], F32, tag="chp", name="c3")
        for mc in range(2):
            nc.tensor.matmul(c3[:, mc, :], bseed[:, :, 2, mc * P:(mc + 1) * P],
                             ones8[:], start=True, stop=False, perf_mode=DR)
        for mc in range(2):
            nc.tensor.matmul(c3[:, mc, :], w8[:, 2, :, mc * P:(mc + 1) * P],
                             h2c[:], start=False, stop=True, perf_mode=DR)
        j = k % 4
        if j == 0:
            ch["coll"] = collp.tile([P, 2, 4, P], FP16, tag="coll",
                                    name="coll")
        coll = ch["coll"]
        ytn = coll[:, :, j, :]
        nc.vector.scalar_tensor_tensor(ytn, c3[:], h8, ytp,
                                       ALU.mult, ALU.add)
        ch["prev"] = ytn
        if j == 3 or k == NSTEPS - 1:
            t0 = T_OBS + 1 + (k // 4) * 4
            nc.sync.dma_start(oTd[:, :, t0:t0 + j + 1, :], coll[:, :, 0:j + 1, :])

    # ---- schedule: PE warmup, then one software-pipelined slot sequence.
    # Slot order starts with the W=4 group (frames 96-99) since the chain
    # hangs off its last output; each slot runs the previous group's L3+store
    # between its L1 and L2 (dependency-free there), with chain steps filling
    # the two act-latency windows. ----
    warm = chps.tile([P, 2, P], F32, tag="chp", name="warm")
    for i in range(10):
        nc.tensor.matmul(warm[:, i % 2, :], bseed[:, :, 0, 0:P], ones8[:],
                         start=True, stop=True, perf_mode=DR)
    wsink = hcp.tile([P, 2, P], FP8, tag="wsink")
    nc.scalar.activation(wsink[:], warm[:], AF.Tanh)

    state = {"k": 0}

    def fill():
        if ch["prev"] is not None and state["k"] < NSTEPS:
            chain_step(state["k"])
            state["k"] += 1

    group(NG_FULL, prefetch=0)            # frames 96..99 -> out 97..100
    pend = {"l3": None}
    for g in range(NG_FULL):
        group(g, mid1=fill, mid2=fill,
              prefetch=g + 1 if g + 1 < NG_FULL else None, pend=pend)
    pend["l3"]()
    while state["k"] < NSTEPS:
        fill()


def _build(h):
    import concourse.mybir as mybir
    import concourse.tile as tile
    from concourse import bacc

    F32 = mybir.dt.float32
    FP16 = mybir.dt.float16
    FP8 = mybir.dt.float8e4

    nc = bacc.Bacc("TRN2", target_bir_lowering=False, debug=False,
                   num_devices=NCORES)
    xT8d = nc.dram_tensor("xT8", [P, 2, T_OBS, P], FP8,
                          kind="ExternalInput").ap()
    xT16d = nc.dram_tensor("xT16", [P, 2, T_OBS, P], FP16,
                           kind="ExternalInput").ap()
    w8d = nc.dram_tensor("w8", [P, 3, 2, D], FP8, kind="ExternalInput").ap()
    bactd = nc.dram_tensor("bact", [P, 4], F32, kind="ExternalInput").ap()
    bseedd = nc.dram_tensor("bseed", [1, 2, 3, D], FP8,
                            kind="ExternalInput").ap()
    ones8d = nc.dram_tensor("ones8", [1, 2, P], FP8, kind="ExternalInput").ap()
    w16d = nc.dram_tensor("w16", [P, 2, D], FP16, kind="ExternalInput").ap()
    b16d = nc.dram_tensor("b16", [1, D + P], FP16, kind="ExternalInput").ap()
    oTd = nc.dram_tensor("oT", [P, 2, T, P], FP16, kind="ExternalOutput").ap()

    with tile.TileContext(nc) as tc, ExitStack() as ctx:
        _emit(ctx, tc, xT8d, xT16d, w8d, bactd, bseedd, ones8d, w16d, b16d,
              oTd, h)
    nc.compile()
    return nc


def _host_inputs(inputs):
    """Shared (weights/bias) device arrays + h. Returns (h, shared)."""
    ts = np.asarray(inputs["time_steps"], np.float32)
    h = float(np.float32(ts[1]) - np.float32(ts[0]))

    f8 = ml_dtypes.float8_e4m3
    W1 = np.asarray(inputs["W1"], np.float32)
    W2 = np.asarray(inputs["W2"], np.float32)
    W3 = np.asarray(inputs["W3"], np.float32)
    b1 = np.asarray(inputs["b1"], np.float32)
    b2 = np.asarray(inputs["b2"], np.float32)
    b3 = np.asarray(inputs["b3"], np.float32)

    # w8[p, wi, kc, m] = 8*W_wi[kc*128+p, m]
    w8 = np.stack([8.0 * W1, 8.0 * W2, 8.0 * W3])  # [3, 256, 256]
    w8 = w8.reshape(3, 2, P, D).transpose(2, 0, 1, 3)  # [p, 3, kc, m]
    w8 = np.ascontiguousarray(w8).astype(f8)

    bact = np.stack([b1[:P], b1[P:], b2[:P], b2[P:]], axis=1)
    bact = np.ascontiguousarray(bact.astype(np.float32))

    # bseed[0, kc, wi, m] = 4*b_wi[m]  (K=2 DR ones contraction doubles it)
    bs = np.stack([4.0 * b1, 4.0 * b2, 4.0 * b3])  # [3, 256]
    bseed = np.broadcast_to(bs[None, None], (1, 2, 3, D))
    bseed = np.ascontiguousarray(bseed).astype(f8)
    ones8 = np.ones((1, 2, P), np.float32).astype(f8)

    # chain L1 runs in fp16: exact W1/b1 (unscaled) + a ones row
    w16 = W1.reshape(2, P, D).transpose(1, 0, 2)   # [p, kc, m]
    w16 = np.ascontiguousarray(w16).astype(np.float16)
    b16 = np.concatenate([b1, np.ones(P, np.float32)]).reshape(1, D + P)
    b16 = b16.astype(np.float16)

    shared = dict(w8=w8, bact=bact, bseed=bseed, ones8=ones8,
                  w16=w16, b16=b16)
    return h, shared


def make_in_maps(inputs):
    """Full per-core input maps (shared + per-core transposed latents)."""
    h, shared = _host_inputs(inputs)
    b3 = np.asarray(inputs["b3"], np.float32)
    lat = np.ascontiguousarray(np.asarray(inputs["latents"], np.float32))
    f8 = ml_dtypes.float8_e4m3

    in_maps = []
    for c in range(NCORES):
        lc = lat[c * PB:(c + 1) * PB]                  # [128b, 100t, 256d]
        xt = lc.transpose(2, 1, 0)                     # [256d, 100t, 128b]
        xt8 = xt.reshape(2, P, T_OBS, P).transpose(1, 0, 2, 3)
        xt16 = (xt + (np.float32(h) * b3)[:, None, None])
        xt16 = xt16.reshape(2, P, T_OBS, P).transpose(1, 0, 2, 3)
        m = dict(shared)
        m["xT8"] = np.ascontiguousarray(xt8).astype(f8)
        m["xT16"] = np.ascontiguousarray(xt16).astype(np.float16)
        in_maps.append(m)
    return h, in_maps


def assemble_out(inputs, core_outs):
    """De-transpose per-core oT outputs and patch the exact copy frames."""
    lat = np.asarray(inputs["latents"], np.float32)
    out = np.empty((B, T, D), np.float32)
    for c in range(NCORES):
        oT = np.asarray(core_outs[c], np.float32)      # [128p, 2dc, 120t, 128b]
        out[c * PB:(c + 1) * PB] = oT.transpose(3, 2, 1, 0).reshape(PB, T, D)
    out[:, 0, :] = lat[:, 0, :]
    out[:, 2, :] = lat[:, 1, :]
    return out


_CACHE = {}


def kernel(**inputs):
    from concourse.bass_utils import run_bass_kernel_spmd

    h, in_maps = make_in_maps(inputs)
    if h not in _CACHE:
        _CACHE[h] = _build(h)
    nc = _CACHE[h]

    res = run_bass_kernel_spmd(nc, in_maps, list(range(NCORES)))
    outs = [res.results[c]["oT"] for c in range(NCORES)]
    return assemble_out(inputs, outs)


# revision 38
# speedup vs baseline: 1.0418x; 1.0002x over previous
"""Trainium2 Bass kernel for nn_ODE_71743133713072 (v2).

Semantics (unrolled from the reference lax.scan; time_steps = linspace, all
nonzero gaps equal h = ts[1]-ts[0]):
  out[:, 0]   = lat[:, 0]
  out[:, 2]   = lat[:, 1]                     (the scan's zero-length gap)
  out[:, t+1] = lat[:, t] + h * f(lat[:, t])  for t = 0..99, t != 1
  y = out[:, 100]
  out[:, k+1] = y = y + h * f(y)              for k = 100..118
where f is the D->U->U->D tanh MLP.

Layout strategy: everything on device lives FEATURE-ON-PARTITION
(transposed). The host pre-transposes the latents into
  xT8  [128p, 2dc, 100t, 128b] fp8   (matmul operand, unscaled)
  xT16 [128p, 2dc, 100t, 128b] fp16  (Euler base, with h*b3 pre-added)
and the device writes the output transposed AND in fp16
(oT [128p, 2dc, 120t, 128b]); the host de-transposes/upcasts after
gather. This removes every PE transpose, every on-device cast, and
every bias matmul from the parallel part (b1/b2 ride along as
per-partition act biases, b3*h is folded into xT16), and halves the
output DMA. Matmuls are fp8 DoubleRow (K=256 in one pass, 0.5
cycles/col, 216 ns/inst steady at the sustained 1.2 GHz PE clock);
LDWEIGHTS pipelines behind the previous matmul.

Scheduling (the key to keeping the PE p-state warm): groups of 8 frames
are software-pipelined — each slot runs L1(g), then the PREVIOUS
group's L3+Euler+store (dependency-free here), then L2(g), with one
19-step-chain step embedded in each of the two act-latency windows so
the tensor queue never drains. The chain keeps a transposed fp16 carry
(the collector slice doubles as carry and DMA source), runs L1 in fp16
straight off the carry (no cast hop), L2/L3 in fp8-DR with K<=2 PSUM
bias seeds, and batches 4 output frames per DMA.

PSUM budget (8 banks): 3 rotating [128,1024]-f32 matmul tiles (6) +
2 chain tiles (2). Measured: ~106-110 us on 8 cores (baseline 232 us),
rel err 1.5e-3 (fp8 weights + fp16 carry/out, tolerance 2e-2).
"""

import os
import sys
from contextlib import ExitStack

import numpy as np

for _p in ("/opt/trn_rl_repo", "/root/.axon_site/_ro/trn_rl_repo"):
    if os.path.isdir(_p) and _p not in sys.path:
        sys.path.append(_p)

import ml_dtypes  # noqa: E402

B, T_OBS, KPRED, D = 1024, 100, 20, 256
T = T_OBS + KPRED          # 120
NCORES = 8
PB = B // NCORES           # 128 rows per core
P = 128
W = 8                      # frames per full group
NG_FULL = 12               # 12 full groups of 8 = 96 frames
W_LAST = 4                 # +1 group of 4 (frames 96..99)
NSTEPS = T - 1 - T_OBS     # 19 chain steps


def _emit(ctx, tc, xT8d, xT16d, w8d, bactd, bseedd, ones8d, w16d, b16d,
          oTd, h):
    import concourse.mybir as mybir

    nc = tc.nc
    F32 = mybir.dt.float32
    FP16 = mybir.dt.float16
    FP8 = mybir.dt.float8e4
    AF = mybir.ActivationFunctionType
    ALU = mybir.AluOpType
    DR = mybir.MatmulPerfMode.DoubleRow

    h8 = float(h / 8.0)

    const = ctx.enter_context(tc.tile_pool(name="const", bufs=1))
    w8 = const.tile([P, 3, 2, D], FP8, tag="w8")
    nc.sync.dma_start(w8[:], w8d[:])
    bact = const.tile([P, 4], F32, tag="bact")
    nc.sync.dma_start(bact[:], bactd[:])
    bseed = const.tile([1, 2, 3, D], FP8, tag="bseed")
    nc.sync.dma_start(bseed[:], bseedd[:])
    ones8 = const.tile([1, 2, P], FP8, tag="ones8")
    nc.sync.dma_start(ones8[:], ones8d[:])
    w16 = const.tile([P, 2, D], FP16, tag="w16")
    nc.sync.dma_start(w16[:], w16d[:])
    b16 = const.tile([1, D + P], FP16, tag="b16")  # [b1 (256) | ones (128)]
    nc.sync.dma_start(b16[:], b16d[:])

    x16p = ctx.enter_context(tc.tile_pool(name="x16", bufs=3))
    x8p = ctx.enter_context(tc.tile_pool(name="x8", bufs=3))
    hp = ctx.enter_context(tc.tile_pool(name="hact", bufs=4))
    oTp = ctx.enter_context(tc.tile_pool(name="oT", bufs=3))
    psp = ctx.enter_context(tc.tile_pool(name="ps", bufs=3, space="PSUM"))

    hcp = ctx.enter_context(tc.tile_pool(name="hc", bufs=4))
    collp = ctx.enter_context(tc.tile_pool(name="coll", bufs=2))
    chps = ctx.enter_context(tc.tile_pool(name="chps", bufs=2, space="PSUM"))

    def g_load(g):
        """DMA-in for group g; returns (x16, x8)."""
        w = W if g < NG_FULL else W_LAST
        t0 = g * W
        x16 = x16p.tile([P, 2, W, P], FP16, tag="x16")
        nc.sync.dma_start(x16[:, :, 0:w, :], xT16d[:, :, t0:t0 + w, :])
        x8 = x8p.tile([P, 2, W, P], FP8, tag="x8")
        nc.sync.dma_start(x8[:, :, 0:w, :], xT8d[:, :, t0:t0 + w, :])
        return x16, x8

    def g_layer(g, wi, rhs_of, out_fp8):
        """One MLP layer for group g: 2mc x w/4 matmuls + per-mc tanh."""
        w = W if g < NG_FULL else W_LAST
        nw = w * P
        mm = [psp.tile([P, W * P], F32, tag="ps", name="mm") for _ in range(2)]
        for mc in range(2):
            for q in range(w // 4):
                nc.tensor.matmul(
                    mm[mc][:, q * 512:(q + 1) * 512],
                    w8[:, wi, :, mc * P:(mc + 1) * P],
                    rhs_of(q), start=True, stop=True, perf_mode=DR)
        if out_fp8 is not None:
            for mc in range(2):
                nc.scalar.activation(out_fp8[:, mc, 0:nw], mm[mc][:, 0:nw],
                                     AF.Tanh, bias=bact[:, 2 * wi + mc:
                                                        2 * wi + mc + 1],
                                     scale=0.125)
        return mm

    def g_store(g, fT, x16):
        """Euler update + DMA-out for group g."""
        w = W if g < NG_FULL else W_LAST
        t0 = g * W
        nw = w * P
        oT = oTp.tile([P, 2, W, P], FP16, tag="oT")
        for mc in range(2):
            nc.vector.scalar_tensor_tensor(
                oT[:, mc, 0:w, :].rearrange("p a b -> p (a b)"),
                fT[mc][:, 0:nw], h8,
                x16[:, mc, 0:w, :].rearrange("p a b -> p (a b)"),
                ALU.mult, ALU.add)
            nc.sync.dma_start(oTd[:, mc, t0 + 1:t0 + w + 1, :],
                              oT[:, mc, 0:w, :])
        if g == NG_FULL:
            # chain carry init: y0 = out[:, 100] (already fp16)
            ch["prev"] = oT[:, :, W_LAST - 1, :]
        return oT

    loads = {}

    def group(g, mid1=None, mid2=None, prefetch=None, pend=None):
        """Group slot. With `pend`, L3+store of the previous group runs here
        (between L1 and L2, where it is dependency-free), and this group's
        own L3 closure is returned via pend for the next slot."""
        x16, x8 = loads.pop(g, None) or g_load(g)
        h1 = hp.tile([P, 2, W * P], FP8, tag="h1")
        h2 = hp.tile([P, 2, W * P], FP8, tag="h2")
        g_layer(g, 0, lambda q: x8[:, :, 4 * q:4 * q + 4, :], h1)
        if prefetch is not None and prefetch not in loads:
            loads[prefetch] = g_load(prefetch)
        if mid1 is not None:
            mid1()
        if pend is not None and pend["l3"] is not None:
            pend["l3"]()
        g_layer(g, 1, lambda q: h1[:, :, q * 512:(q + 1) * 512], h2)
        if mid2 is not None:
            mid2()
        if pend is None:
            fT = g_layer(g, 2, lambda q: h2[:, :, q * 512:(q + 1) * 512], None)
            return g_store(g, fT, x16)

        def do_l3(g=g, h2=h2, x16=x16):
            fT = g_layer(g, 2, lambda q: h2[:, :, q * 512:(q + 1) * 512], None)
            g_store(g, fT, x16)
        pend["l3"] = do_l3
        return None

    # chain state: carry slices; coll tiles batch 4 output frames per DMA
    ch = {"prev": None, "coll": None}

    def chain_step(k):
        """out[:,100+k+1] = y + h*f(y); y is the transposed fp16 carry."""
        ytp = ch["prev"]   # [P, 2, P] fp16
        # L1 in fp16 straight off the carry (no cast hop): exact b1 seed
        # (K=1) + two kc-accumulated fp16 matmuls per mc chunk.
        c1 = chps.tile([P, 2, P], F32, tag="chp", name="c1")
        for mc in range(2):
            nc.tensor.matmul(c1[:, mc, :], b16[:, mc * P:(mc + 1) * P],
                             b16[:, D:D + P], start=True, stop=False)
        for mc in range(2):
            for kc in range(2):
                nc.tensor.matmul(c1[:, mc, :],
                                 w16[:, kc, mc * P:(mc + 1) * P],
                                 ytp[:, kc, :], start=False, stop=(kc == 1))
        h1c = hcp.tile([P, 2, P], FP8, tag="h1c")
        nc.scalar.activation(h1c[:], c1[:], AF.Tanh)
        c2 = chps.tile([P, 2, P], F32, tag="chp", name="c2")
        for mc in range(2):
            nc.tensor.matmul(c2[:, mc, :], bseed[:, :, 1, mc * P:(mc + 1) * P],
                             ones8[:], start=True, stop=False, perf_mode=DR)
        for mc in range(2):
            nc.tensor.matmul(c2[:, mc, :], w8[:, 1, :, mc * P:(mc + 1) * P],
                             h1c[:], start=False, stop=True, perf_mode=DR)
        h2c = hcp.tile([P, 2, P], FP8, tag="h2c")
        nc.scalar.activation(h2c[:], c2[:], AF.Tanh, scale=0.125)
        c3 = chps.tile([P, 2, P], F32, tag="chp", name="c3")
        for mc in range(2):
            nc.tensor.matmul(c3[:, mc, :], bseed[:, :, 2, mc * P:(mc + 1) * P],
                             ones8[:], start=True, stop=False, perf_mode=DR)
        for mc in range(2):
            nc.tensor.matmul(c3[:, mc, :], w8[:, 2, :, mc * P:(mc + 1) * P],
                             h2c[:], start=False, stop=True, perf_mode=DR)
        j = k % 4
        if j == 0:
            ch["coll"] = collp.tile([P, 2, 4, P], FP16, tag="coll",
                                    name="coll")
        coll = ch["coll"]
        ytn = coll[:, :, j, :]
        nc.vector.scalar_tensor_tensor(ytn, c3[:], h8, ytp,
                                       ALU.mult, ALU.add)
        ch["prev"] = ytn
        if j == 3 or k == NSTEPS - 1:
            t0 = T_OBS + 1 + (k // 4) * 4
            nc.sync.dma_start(oTd[:, :, t0:t0 + j + 1, :], coll[:, :, 0:j + 1, :])

    # ---- schedule: PE warmup, then one software-pipelined slot sequence.
    # Slot order starts with the W=4 group (frames 96-99) since the chain
    # hangs off its last output; each slot runs the previous group's L3+store
    # between its L1 and L2 (dependency-free there), with chain steps filling
    # the two act-latency windows. ----
    warm = chps.tile([P, 2, P], F32, tag="chp", name="warm")
    for i in range(10):
        nc.tensor.matmul(warm[:, i % 2, :], bseed[:, :, 0, 0:P], ones8[:],
                         start=True, stop=True, perf_mode=DR)
    wsink = hcp.tile([P, 2, P], FP8, tag="wsink")
    nc.scalar.activation(wsink[:], warm[:], AF.Tanh)

    state = {"k": 0}

    def fill():
        if ch["prev"] is not None and state["k"] < NSTEPS:
            chain_step(state["k"])
            state["k"] += 1

    group(NG_FULL, prefetch=0)            # frames 96..99 -> out 97..100
    pend = {"l3": None}
    for g in range(NG_FULL):
        group(g, mid1=fill, mid2=fill,
              prefetch=g + 1 if g + 1 < NG_FULL else None, pend=pend)
    pend["l3"]()
    while state["k"] < NSTEPS:
        fill()


def _build(h):
    import concourse.mybir as mybir
    import concourse.tile as tile
    from concourse import bacc

    F32 = mybir.dt.float32
    FP16 = mybir.dt.float16
    FP8 = mybir.dt.float8e4

    nc = bacc.Bacc("TRN2", target_bir_lowering=False, debug=False,
                   num_devices=NCORES)
    xT8d = nc.dram_tensor("xT8", [P, 2, T_OBS, P], FP8,
                          kind="ExternalInput").ap()
    xT16d = nc.dram_tensor("xT16", [P, 2, T_OBS, P], FP16,
                           kind="ExternalInput").ap()
    w8d = nc.dram_tensor("w8", [P, 3, 2, D], FP8, kind="ExternalInput").ap()
    bactd = nc.dram_tensor("bact", [P, 4], F32, kind="ExternalInput").ap()
    bseedd = nc.dram_tensor("bseed", [1, 2, 3, D], FP8,
                            kind="ExternalInput").ap()
    ones8d = nc.dram_tensor("ones8", [1, 2, P], FP8, kind="ExternalInput").ap()
    w16d = nc.dram_tensor("w16", [P, 2, D], FP16, kind="ExternalInput").ap()
    b16d = nc.dram_tensor("b16", [1, D + P], FP16, kind="ExternalInput").ap()
    oTd = nc.dram_tensor("oT", [P, 2, T, P], FP16, kind="ExternalOutput").ap()

    with tile.TileContext(nc) as tc, ExitStack() as ctx:
        _emit(ctx, tc, xT8d, xT16d, w8d, bactd, bseedd, ones8d, w16d, b16d,
              oTd, h)
    nc.compile()
    return nc


def _host_inputs(inputs):
    """Shared (weights/bias) device arrays + h. Returns (h, shared)."""
    ts = np.asarray(inputs["time_steps"], np.float32)
    h = float(np.float32(ts[1]) - np.float32(ts[0]))

    f8 = ml_dtypes.float8_e4m3
    W1 = np.asarray(inputs["W1"], np.float32)
    W2 = np.asarray(inputs["W2"], np.float32)
    W3 = np.asarray(inputs["W3"], np.float32)
    b1 = np.asarray(inputs["b1"], np.float32)
    b2 = np.asarray(inputs["b2"], np.float32)
    b3 = np.asarray(inputs["b3"], np.float32)

    # w8[p, wi, kc, m] = 8*W_wi[kc*128+p, m]
    w8 = np.stack([8.0 * W1, 8.0 * W2, 8.0 * W3])  # [3, 256, 256]
    w8 = w8.reshape(3, 2, P, D).transpose(2, 0, 1, 3)  # [p, 3, kc, m]
    w8 = np.ascontiguousarray(w8).astype(f8)

    bact = np.stack([b1[:P], b1[P:], b2[:P], b2[P:]], axis=1)
    bact = np.ascontiguousarray(bact.astype(np.float32))

    # bseed[0, kc, wi, m] = 4*b_wi[m]  (K=2 DR ones contraction doubles it)
    bs = np.stack([4.0 * b1, 4.0 * b2, 4.0 * b3])  # [3, 256]
    bseed = np.broadcast_to(bs[None, None], (1, 2, 3, D))
    bseed = np.ascontiguousarray(bseed).astype(f8)
    ones8 = np.ones((1, 2, P), np.float32).astype(f8)

    # chain L1 runs in fp16: exact W1/b1 (unscaled) + a ones row
    w16 = W1.reshape(2, P, D).transpose(1, 0, 2)   # [p, kc, m]
    w16 = np.ascontiguousarray(w16).astype(np.float16)
    b16 = np.concatenate([b1, np.ones(P, np.float32)]).reshape(1, D + P)
    b16 = b16.astype(np.float16)

    shared = dict(w8=w8, bact=bact, bseed=bseed, ones8=ones8,
                  w16=w16, b16=b16)
    return h, shared


def make_in_maps(inputs):
    """Full per-core input maps (shared + per-core transposed latents)."""
    h, shared = _host_inputs(inputs)
    b3 = np.asarray(inputs["b3"], np.float32)
    lat = np.ascontiguousarray(np.asarray(inputs["latents"], np.float32))
    f8 = ml_dtypes.float8_e4m3

    in_maps = []
    for c in range(NCORES):
        lc = lat[c * PB:(c + 1) * PB]                  # [128b, 100t, 256d]
        xt = lc.transpose(2, 1, 0)                     # [256d, 100t, 128b]
        xt8 = xt.reshape(2, P, T_OBS, P).transpose(1, 0, 2, 3)
        xt16 = (xt + (np.float32(h) * b3)[:, None, None])
        xt16 = xt16.reshape(2, P, T_OBS, P).transpose(1, 0, 2, 3)
        m = dict(shared)
        m["xT8"] = np.ascontiguousarray(xt8).astype(f8)
        m["xT16"] = np.ascontiguousarray(xt16).astype(np.float16)
        in_maps.append(m)
    return h, in_maps


def assemble_out(inputs, core_outs):
    """De-transpose per-core oT outputs and patch the exact copy frames."""
    lat = np.asarray(inputs["latents"], np.float32)
    out = np.empty((B, T, D), np.float32)
    for c in range(NCORES):
        oT = np.asarray(core_outs[c], np.float32)      # [128p, 2dc, 120t, 128b]
        out[c * PB:(c + 1) * PB] = oT.transpose(3, 2, 1, 0).reshape(PB, T, D)
    out[:, 0, :] = lat[:, 0, :]
    out[:, 2, :] = lat[:, 1, :]
    return out


_CACHE = {}


def kernel(**inputs):
    from concourse.bass_utils import run_bass_kernel_spmd

    h, in_maps = make_in_maps(inputs)
    if h not in _CACHE:
        _CACHE[h] = _build(h)
    nc = _CACHE[h]

    res = run_bass_kernel_spmd(nc, in_maps, list(range(NCORES)))
    outs = [res.results[c]["oT"] for c in range(NCORES)]
    return assemble_out(inputs, outs)
